# revision 2
# baseline (speedup 1.0000x reference)
"""Trainium2 Bass kernel for the GCNN layer (nn_GCNNLayer_71536975282326).

out = relu( einsum('nd,nde->ne', x, W_pos) + b_pos
            + einsum('nre,nr->ne', einsum('nd,rde->nre', x, W_dep), counts)
            + counts @ b_dep )
with counts[n,r] = #edges (token n, type r).

Strategy (8 NeuronCores, SPMD, one program):
  - Shard the R=92 W_dep stack across cores (12 slots/core, zero-padded) and
    the N=150 W_pos stack across cores (19 slots/core, zero-padded).
  - Everything is computed transposed: out_T[e, n], accumulated in PSUM.
    PSUM: 8 banks, one per 128-wide e-chunk, each [128, 150+19].
      cols 0:150   -> dep partial for ALL tokens + bias terms
      cols 150:169 -> self term for this core's LOCAL tokens
  - Dep slot r: 64 matmuls  W_r[d,e]^T (stationary) @ (counts[:,r]*x)^T (moving).
    The scaled moving operand is precomputed on host (tiny vs the 4MB matrix).
  - Self slot j: 64 matvecs W_pos[n_j][d,e]^T @ x_{n_j}^T into PSUM col 150+j.
  - Bias: one K=32 matmul per e-chunk: lhsT rows = [b_dep slice ; b_pos rows],
    rhs = [counts slice^T ; one-hot placing token n_j at global column n_j].
  - AllReduce the [1024,150] main partial; AllGather the [1024,19] self
    partials (slot (core k, j) IS global token 19k+j, so reassembly is a plain
    3D DMA); add + relu on device; host transposes the [1024,150] result.
"""

import numpy as np

import concourse.bass as bass
import concourse.tile as tile
from concourse import bacc, mybir
from concourse.bass_utils import run_bass_kernel_spmd

N, D, R = 150, 1024, 92
NCORES = 8
P = 128
DC = D // P            # 8 contraction (d) chunks
EC = D // P            # 8 output (e) chunks
DEP_SLOTS = 12         # ceil(92/8)
SELF_SLOTS = 19        # ceil(150/8)
KAUG = 32              # 12 dep-count rows + 19 one-hot rows + 1 pad
F32 = mybir.dt.float32

DEP_SPLIT = [12, 12, 12, 12, 11, 11, 11, 11]
DEP_STARTS = np.concatenate([[0], np.cumsum(DEP_SPLIT)])

_PROG = None


def _unit_sequence():
    """Interleave 19 self units among 12 dep units (dep are PE-heavy, self are
    PE-light) so the tensor engine never falls far behind the DMA stream.
    Ends with a self unit to keep the PE tail short."""
    units = []
    a = b = 0
    while a < SELF_SLOTS or b < DEP_SLOTS:
        if a < SELF_SLOTS and (b >= DEP_SLOTS or a * DEP_SLOTS <= b * SELF_SLOTS):
            units.append(("self", a))
            a += 1
        else:
            units.append(("dep", b))
            b += 1
    if units[-1][0] == "dep":
        units[-1], units[-2] = units[-2], units[-1]
    return units


def _build_program():
    nc = bacc.Bacc("TRN2", target_bir_lowering=False, debug=False, num_devices=NCORES)

    wdep = nc.dram_tensor("wdep", [DEP_SLOTS, D, D], F32, kind="ExternalInput")
    wpos = nc.dram_tensor("wpos", [SELF_SLOTS, D, D], F32, kind="ExternalInput")
    xs = nc.dram_tensor("xs", [DEP_SLOTS, DC, P, N], F32, kind="ExternalInput")
    xtl = nc.dram_tensor("xtl", [DC, P, SELF_SLOTS], F32, kind="ExternalInput")
    baug = nc.dram_tensor("baug", [KAUG, D], F32, kind="ExternalInput")
    caug = nc.dram_tensor("caug", [KAUG, N], F32, kind="ExternalInput")
    out_T = nc.dram_tensor("out_T", [D, N], F32, kind="ExternalOutput")

    groups = [list(range(NCORES))]
    units = _unit_sequence()
    W = N + SELF_SLOTS  # psum tile width

    with tile.TileContext(nc) as tc:
        with (
            tc.tile_pool(name="wpool", bufs=4) as wpool,
            tc.tile_pool(name="xspool", bufs=3) as xspool,
            tc.tile_pool(name="constp", bufs=1) as constp,
            tc.tile_pool(name="psp", bufs=1, space=bass.MemorySpace.PSUM) as psp,
            tc.tile_pool(name="dram", bufs=1, space="DRAM") as dram,
            tc.tile_pool(name="fin", bufs=3) as fin,
        ):
            xtl_t = constp.tile([P, DC * SELF_SLOTS], F32)
            nc.sync.dma_start(
                out=xtl_t.rearrange("p (c j) -> p c j", c=DC),
                in_=xtl[:].rearrange("c p j -> p c j"),
            )
            baug_t = constp.tile([KAUG, D], F32)
            nc.sync.dma_start(out=baug_t[:], in_=baug[:])
            caug_t = constp.tile([KAUG, N], F32)
            nc.sync.dma_start(out=caug_t[:], in_=caug[:])

            accs = [
                psp.tile([P, W], F32, name=f"acc{ec}", tag=f"acc{ec}")
                for ec in range(EC)
            ]
            # Bias matmuls come first: the single start=True per PSUM bank.
            for ec in range(EC):
                nc.tensor.matmul(
                    accs[ec][:, 0:N],
                    baug_t[:, ec * P : (ec + 1) * P],
                    caug_t[:],
                    start=True,
                    stop=False,
                )

            for u, (kind, idx) in enumerate(units):
                wt = wpool.tile([P, DC * D], F32, tag="w", name=f"w{u}")
                src3 = (wdep if kind == "dep" else wpos)[idx].rearrange(
                    "(c p) e -> p c e", p=P
                )
                wt3 = wt.rearrange("p (c e) -> p c e", c=DC)
                for g in range(4):
                    nc.sync.dma_start(
                        out=wt3[:, 2 * g : 2 * g + 2, :],
                        in_=src3[:, 2 * g : 2 * g + 2, :],
                    )
                last = u == len(units) - 1
                if kind == "dep":
                    xst = xspool.tile([P, DC * N], F32, tag="xs", name=f"xs{u}")
                    nc.sync.dma_start(
                        out=xst.rearrange("p (c f) -> p c f", c=DC),
                        in_=xs[idx].rearrange("c p f -> p c f"),
                    )
                    for c in range(DC):
                        for ec in range(EC):
                            nc.tensor.matmul(
                                accs[ec][:, 0:N],
                                wt[:, c * D + ec * P : c * D + (ec + 1) * P],
                                xst[:, c * N : (c + 1) * N],
                                start=False,
                                stop=last and c == DC - 1,
                            )
                else:
                    j = idx
                    for c in range(DC):
                        for ec in range(EC):
                            nc.tensor.matmul(
                                accs[ec][:, N + j : N + j + 1],
                                wt[:, c * D + ec * P : c * D + (ec + 1) * P],
                                xtl_t[:, c * SELF_SLOTS + j : c * SELF_SLOTS + j + 1],
                                start=False,
                                stop=last and c == DC - 1,
                            )

            ar_main_in = dram.tile([D, N], F32)
            ar_main_out = dram.tile([D, N], F32, addr_space="Shared")
            ar_self_in = dram.tile([D, SELF_SLOTS], F32)
            ar_self_out = dram.tile([NCORES, D, SELF_SLOTS], F32, addr_space="Shared")

            for ec in range(EC):
                ev = fin.tile([P, W], F32, tag="ev", name=f"ev{ec}")
                nc.vector.tensor_copy(ev[:], accs[ec][:])
                nc.sync.dma_start(out=ar_main_in[ec * P : (ec + 1) * P, :], in_=ev[:, 0:N])
                nc.sync.dma_start(out=ar_self_in[ec * P : (ec + 1) * P, :], in_=ev[:, N:W])

            nc.gpsimd.collective_compute(
                "AllReduce", mybir.AluOpType.add,
                replica_groups=groups, ins=[ar_main_in.opt()], outs=[ar_main_out.opt()],
            )
            nc.gpsimd.collective_compute(
                "AllGather", mybir.AluOpType.bypass,
                replica_groups=groups, ins=[ar_self_in.opt()], outs=[ar_self_out.opt()],
            )

            for ec in range(EC):
                mc = fin.tile([P, N], F32, tag="mc", name=f"mc{ec}")
                nc.sync.dma_start(out=mc[:], in_=ar_main_out[ec * P : (ec + 1) * P, :])
                sc = fin.tile([P, NCORES * SELF_SLOTS], F32, tag="sc", name=f"sc{ec}")
                nc.sync.dma_start(
                    out=sc.rearrange("p (k j) -> p k j", k=NCORES),
                    in_=ar_self_out[:, ec * P : (ec + 1) * P, :].rearrange("k p j -> p k j"),
                )
                oc = fin.tile([P, N], F32, tag="oc", name=f"oc{ec}")
                # oc = relu(mc + sc[:, 0:150]); self slot (k, j) == global token 19k+j
                nc.vector.scalar_tensor_tensor(
                    oc[:], mc[:], 0.0, sc[:, 0:N],
                    mybir.AluOpType.add, mybir.AluOpType.add,
                )
                nc.vector.tensor_scalar_max(oc[:], oc[:], 0.0)
                nc.sync.dma_start(out=out_T[ec * P : (ec + 1) * P, :], in_=oc[:])

    nc.compile()
    return nc


def _get_program():
    global _PROG
    if _PROG is None:
        _PROG = _build_program()
    return _PROG


def _prepare_in_maps(x, W_pos, b_pos, W_dep, b_dep, edge_token, edge_type):
    x = np.ascontiguousarray(np.asarray(x, dtype=np.float32))
    W_pos = np.asarray(W_pos, dtype=np.float32)
    b_pos = np.asarray(b_pos, dtype=np.float32)
    W_dep = np.asarray(W_dep, dtype=np.float32)
    b_dep = np.asarray(b_dep, dtype=np.float32)
    edge_token = np.asarray(edge_token)
    edge_type = np.asarray(edge_type)

    counts = np.zeros((N, R), np.float32)
    np.add.at(counts, (edge_token, edge_type), 1.0)
    xT = np.ascontiguousarray(x.T)  # [D, N]

    in_maps = []
    for k in range(NCORES):
        r0, r1 = int(DEP_STARTS[k]), int(DEP_STARTS[k + 1])
        nr = r1 - r0
        t0 = SELF_SLOTS * k
        t1 = min(t0 + SELF_SLOTS, N)
        nt = t1 - t0

        wdep_k = np.zeros((DEP_SLOTS, D, D), np.float32)
        wdep_k[:nr] = W_dep[r0:r1]
        wpos_k = np.zeros((SELF_SLOTS, D, D), np.float32)
        wpos_k[:nt] = W_pos[t0:t1]

        xs_k = np.zeros((DEP_SLOTS, DC, P, N), np.float32)
        for i in range(nr):
            xs_k[i] = (xT * counts[:, r0 + i][None, :]).reshape(DC, P, N)

        xtl_k = np.zeros((DC, P, SELF_SLOTS), np.float32)
        xtl_k[:, :, :nt] = xT[:, t0:t1].reshape(DC, P, nt)

        baug_k = np.zeros((KAUG, D), np.float32)
        baug_k[:nr] = b_dep[r0:r1]
        baug_k[DEP_SLOTS : DEP_SLOTS + nt] = b_pos[t0:t1]

        caug_k = np.zeros((KAUG, N), np.float32)
        caug_k[:nr] = counts[:, r0:r1].T
        for j in range(nt):
            caug_k[DEP_SLOTS + j, t0 + j] = 1.0

        in_maps.append(
            dict(wdep=wdep_k, wpos=wpos_k, xs=xs_k, xtl=xtl_k, baug=baug_k, caug=caug_k)
        )
    return in_maps


def _run(in_maps, trace=False):
    nc = _get_program()
    return run_bass_kernel_spmd(nc, in_maps, list(range(NCORES)), trace=trace)


def kernel(x, W_pos, b_pos, W_dep, b_dep, edge_token, edge_type):
    in_maps = _prepare_in_maps(x, W_pos, b_pos, W_dep, b_dep, edge_token, edge_type)
    res = _run(in_maps, trace=False)
    return np.ascontiguousarray(res.results[0]["out_T"].T)


def kernel_traced(x, W_pos, b_pos, W_dep, b_dep, edge_token, edge_type):
    """Like kernel() but with NTFF profiling; returns (output, BassKernelResults)."""
    in_maps = _prepare_in_maps(x, W_pos, b_pos, W_dep, b_dep, edge_token, edge_type)
    res = _run(in_maps, trace=True)
    return np.ascontiguousarray(res.results[0]["out_T"].T), res


def install_ntff_shim():
    """The agent image's antenv lacks axon_hooks; recreate it from the boot
    module's ctypes NTFF driver so run_bass_kernel_spmd(trace=True) can
    capture a neuron-profile. Test-only; kernel() never needs this."""
    import sys
    import types

    try:
        from antenv.axon_hooks import get_axon_ntff_profile_hook  # noqa: F401
        return
    except ImportError:
        pass
    from trn_agent_boot.trn_boot import _ntff_profile_via_ctypes

    hook = _ntff_profile_via_ctypes("/opt/axon/libaxon_pjrt.so")
    mod = types.ModuleType("antenv.axon_hooks")
    mod._hook = hook
    mod.get_axon_ntff_profile_hook = lambda: mod._hook
    mod.set_axon_ntff_profile_hook = lambda h: setattr(mod, "_hook", h)
    sys.modules["antenv.axon_hooks"] = mod
